# revision 38
# baseline (speedup 1.0000x reference)
"""CAPAttentionModule Trainium2 kernel, v3 (fp8 DoubleRow, deep interleave).

Data-parallel over batch: 8 images -> 8 NeuronCores, one image per core.
Per core (x: [512, 9216] = [C, H*W], H=W=96), fp8 e4m3 on the PE with
DoubleRow (2x contraction packing):
  k1 = relu(Wkp x + b)            [128, HW]
  v1 = relu(Wvp x + b)            [256, HW]
  k2/v2 = relu(dw3x3(.) + b)      depthwise via diagonal fp8 matmuls:
                                  3 DR pair-matmuls (dy=-1/+1) + 3 singles
  key/value pools: 5D strided DVE reduces -> 24x24 sums -> 1/3/6/8 grids
  q  = relu(Wq x + b)             [256, HW]
  simT = keyn^T q; pe = exp(simT) (keyn pre-scaled by pool-norm/16)
  ctx_u = value @ pe              UNNORMALIZED
Device ships ctx_u (bf16) + pe (bf16); host computes
  y = x + ctx_u / pe.sum(tokens).

Schedule: PE order = k-primary, vlo-primary, dw-k, vhi-primary, then a
48-tick interleave of dw-v row-blocks with q/sim and per-chunk ctx
matmuls (cv streams unblock as value pools finish); ctx stores stream
over sync+scalar DMA queues throughout the second half.
"""

import numpy as np

P = 128
HH = 96
WP = 104          # padded map row pitch (fp8); 2*WP % 16 == 0 for DR pairs
HROWS = 98
MPAD = 16         # guard pad before/after map data (junk reads land here)
MAPN = MPAD + HROWS * WP + MPAD
HW = 9216
S = 110
SP = 112          # padded key free pitch (fp8 DR lhsT pair step % 16 == 0)
NCH = 18
NCW = 512
HALF = HW // 2


def build_bass():
    import concourse.bacc as bacc
    import concourse.tile as tile
    from concourse import mybir
    from concourse.bass import AP
    from contextlib import ExitStack

    f32 = mybir.dt.float32
    bf16 = mybir.dt.bfloat16
    f8 = mybir.dt.float8e4
    DR = mybir.MatmulPerfMode.DoubleRow
    AF = mybir.ActivationFunctionType
    AX = mybir.AxisListType

    nc = bacc.Bacc("TRN2", target_bir_lowering=False, debug=False,
                   enable_asserts=False, num_devices=8)

    x8_d = nc.dram_tensor("x8", [512, HW], f8, kind="ExternalInput").ap()
    wkp_d = nc.dram_tensor("wkp8", [128, 512], f8, kind="ExternalInput").ap()
    wvp_d = nc.dram_tensor("wvp8", [128, 1024], f8, kind="ExternalInput").ap()
    wq_d = nc.dram_tensor("wq8", [128, 1024], f8, kind="ExternalInput").ap()
    dga_d = nc.dram_tensor("dga", [128, 2304], f8, kind="ExternalInput").ap()
    dgs_d = nc.dram_tensor("dgs", [128, 1152], f8, kind="ExternalInput").ap()
    id_d = nc.dram_tensor("ident", [128, 128], bf16, kind="ExternalInput").ap()
    sclk_d = nc.dram_tensor("sclk", [128, 2 * S], f32, kind="ExternalInput").ap()
    sclv_d = nc.dram_tensor("sclv", [128, 4 * S], f32, kind="ExternalInput").ap()
    bias_d = nc.dram_tensor("bias", [128, 8], f32, kind="ExternalInput").ap()
    ctx_d = nc.dram_tensor("ctx", [512, HW], bf16, kind="ExternalOutput").ap()
    pe_d = nc.dram_tensor("pe", [S, HW], bf16, kind="ExternalOutput").ap()

    x8_r = x8_d.rearrange("(t p) n -> p t n", p=P)       # [128, 4, 9216]
    ctx_r = ctx_d.rearrange("(cv p) n -> p cv n", p=P)   # [128, 4, 9216]

    def mk(base_ap, off, dims):
        """Manual AP: keep base's partition entry, custom free dims."""
        return AP(base_ap.tensor, base_ap.offset + off,
                  [list(base_ap.ap[0])] + [[s, n] for s, n in dims])

    with tile.TileContext(nc) as tc:
        with ExitStack() as top:
            cpool = top.enter_context(tc.tile_pool(name="consts", bufs=1))
            kpool = top.enter_context(tc.tile_pool(name="keep", bufs=1))
            tmpp = top.enter_context(tc.tile_pool(name="tmp", bufs=1))

            x8t = [kpool.tile([P, 4 * 1536], f8, name=f"x8_{i6}")
                   for i6 in range(6)]
            nc.sync.dma_start(x8t[0][:].rearrange("p (t n) -> p t n", t=4),
                              x8_r[:, :, 0:1536])
            c_wkp = cpool.tile([P, 512], f8)
            nc.scalar.dma_start(c_wkp[:], wkp_d)
            c_bias = cpool.tile([P, 8], f32)
            nc.scalar.dma_start(c_bias[:], bias_d)
            c_dga = cpool.tile([P, 2304], f8)
            c_dgs = cpool.tile([P, 1152], f8)
            c_wvp = cpool.tile([P, 1024], f8)
            c_wq = cpool.tile([P, 1024], f8)
            c_id = cpool.tile([P, 128], bf16)
            c_sclk = cpool.tile([P, 2 * S], f32)
            c_sclv = cpool.tile([P, 4 * S], f32)
            # prewarm ACT function tables while DMAs land
            warm = cpool.tile([P, 2], f32)
            nc.scalar.activation(warm[:, 0:1], c_bias[:, 0:1], AF.Relu)
            nc.scalar.activation(warm[:, 1:2], c_bias[:, 0:1], AF.Exp)

            maps = [kpool.tile([P, MAPN], f8, name=f"map{i}") for i in range(3)]
            kbm = [kpool.tile([P, HW], bf16, name=f"kb{i}") for i in range(3)]
            rs = kpool.tile([P, 2304], bf16)
            p24 = kpool.tile([P, 6 * 576], f32)
            allp = kpool.tile([P, 6 * S], f32)
            keyn8 = kpool.tile([P, 2 * SP], f8)     # [kq, 112] padded
            valn = kpool.tile([P, 4 * S], bf16)
            vT = kpool.tile([P, 512], bf16)
            pe_all = kpool.tile([P, NCH * NCW], bf16)
            t358 = kpool.tile([P, 408], f32)

            # zero map borders: row 0, row 97, col 0, col 97
            for m in maps:
                mv = mk(m[:], MPAD, [[WP, HROWS], [1, WP]])
                nc.gpsimd.memset(mv[:, 0:1, :], 0.0)
                nc.gpsimd.memset(mv[:, 97:98, :], 0.0)
                nc.gpsimd.memset(mv[:, 1:97, 0:1], 0.0)
                nc.gpsimd.memset(mv[:, 1:97, 97:98], 0.0)
            # zero keyn pad columns (110, 111 of each kq)
            kn = keyn8[:].rearrange("p (o s) -> p o s", o=2)
            nc.gpsimd.memset(kn[:, :, S:SP], 0.0)
            nc.scalar.dma_start(c_dga[:], dga_d)
            nc.scalar.dma_start(c_dgs[:], dgs_d)
            for i6 in range(1, 6):
                nc.sync.dma_start(
                    x8t[i6][:].rearrange("p (t n) -> p t n", t=4),
                    x8_r[:, :, i6 * 1536:(i6 + 1) * 1536])
            nc.scalar.dma_start(c_wvp[:], wvp_d)
            nc.scalar.dma_start(c_wq[:], wq_d)
            nc.scalar.dma_start(c_id[:], id_d)
            nc.scalar.dma_start(c_sclk[:], sclk_d)
            nc.scalar.dma_start(c_sclv[:], sclv_d)

            def xv(lo):
                """(tile, local offset) for pixel range starting at lo."""
                t = x8t[lo // 1536]
                return t[:].rearrange("p (t n) -> p t n", t=4), lo % 1536

            wv_k = c_wkp[:].rearrange("p (kc o m) -> p kc o m", kc=2, o=2)
            wv_v = c_wvp[:].rearrange("p (kc o m) -> p kc o m", kc=2, o=2)
            wv_q = c_wq[:].rearrange("p (kc o m) -> p kc o m", kc=2, o=2)
            dgav = c_dga[:].rearrange("p (ci dx o m) -> p ci dx o m", ci=3, dx=3, o=2)
            dgsv = c_dgs[:].rearrange("p (ci dx m) -> p ci dx m", ci=3, dx=3)

            # stage-2 pools: maps [m0, m1) of p24 -> allp (4 grids each)
            def smallpools(m0, m1):
                m = m1 - m0
                allp_v = allp[:, m0 * S:m1 * S].rearrange("p (m s) -> p m s", s=S)
                p24s = p24[:, m0 * 576:m1 * 576]
                nc.vector.reduce_sum(
                    allp_v[:, :, 0:1],
                    p24s.rearrange("p (m s) -> p m s", s=576), axis=AX.X)
                tmp = tmpp.tile([P, 1152], f32, name="tmp", tag="tmp")
                nc.vector.reduce_sum(
                    tmp[:, 0:m * 72],
                    p24s.rearrange("p (mh wq ws) -> p mh wq ws", wq=3, ws=8),
                    axis=AX.X)
                nc.vector.reduce_sum(
                    allp_v[:, :, 1:10],
                    tmp[:, 0:m * 72].rearrange(
                        "p (m hq hs wq) -> p m hq wq hs", m=m, hq=3, hs=8),
                    axis=AX.X)
                tmp6 = tmpp.tile([P, 1152], f32, name="tmp6", tag="tmp")
                nc.vector.reduce_sum(
                    tmp6[:, 0:m * 144],
                    p24s.rearrange("p (mh wq ws) -> p mh wq ws", wq=6, ws=4),
                    axis=AX.X)
                nc.vector.reduce_sum(
                    allp_v[:, :, 10:46],
                    tmp6[:, 0:m * 144].rearrange(
                        "p (m hq hs wq) -> p m hq wq hs", m=m, hq=6, hs=4),
                    axis=AX.X)
                tmp8 = tmpp.tile([P, 1152], f32, name="tmp8", tag="tmp")
                nc.vector.reduce_sum(
                    tmp8[:, 0:m * 192],
                    p24s.rearrange("p (mh wq ws) -> p mh wq ws", wq=8, ws=3),
                    axis=AX.X)
                nc.vector.reduce_sum(
                    allp_v[:, :, 46:110],
                    tmp8[:, 0:m * 192].rearrange(
                        "p (m hq hs wq) -> p m hq wq hs", m=m, hq=8, hs=3),
                    axis=AX.X)

            def map_pool(mi, slot, r0, r1):
                """4x4 stage-1 pool of map mi, stage-1 rows [r0, r1)."""
                src = mk(maps[mi][:], MPAD + (4 * r0 + 1) * WP + 1,
                         [[4 * WP, r1 - r0], [4, 24], [WP, 4], [1, 4]])
                nc.vector.reduce_sum(
                    p24[:, slot * 576 + r0 * 24:slot * 576 + r1 * 24],
                    src, axis=AX.XY)

            def kb_rowsum(ci, r0, r1):
                """dw-map ci: sum 4 map-rows of each stage-1 row r0..r1 -> rs
                (bf16 all-SBUF adds run in the DVE 2x mode)."""
                n = r1 - r0
                dst = mk(rs[:], r0 * 96, [[1, n * 96]])
                a = [mk(kbm[ci][:], r0 * 384 + i * 96, [[384, n], [1, 96]])
                     for i in range(4)]
                nc.vector.tensor_tensor(dst, a[0], a[1],
                                        op=mybir.AluOpType.add)
                nc.vector.tensor_tensor(dst, dst, a[2],
                                        op=mybir.AluOpType.add)
                nc.vector.tensor_tensor(dst, dst, a[3],
                                        op=mybir.AluOpType.add)

            def kb_colred(ci, r0, r1):
                """col-reduce rs rows r0..r1 into p24 slot for dw map ci."""
                slot = (1, 4, 5)[ci]
                src = mk(rs[:], r0 * 96, [[96, r1 - r0], [4, 24], [1, 4]])
                nc.vector.reduce_sum(
                    p24[:, slot * 576 + r0 * 24:slot * 576 + r1 * 24],
                    src, axis=AX.X)

            def sp5_cols(r0, r1):
                """map slot 5: per-row column pools for grids 3/6/8."""
                sl = p24[:, 5 * 576 + r0 * 24:5 * 576 + r1 * 24]
                nc.vector.reduce_sum(
                    t358[:, r0 * 3:r1 * 3],
                    sl.rearrange("p (mh wq ws) -> p mh wq ws", wq=3, ws=8),
                    axis=AX.X)
                nc.vector.reduce_sum(
                    t358[:, 72 + r0 * 6:72 + r1 * 6],
                    sl.rearrange("p (mh wq ws) -> p mh wq ws", wq=6, ws=4),
                    axis=AX.X)
                nc.vector.reduce_sum(
                    t358[:, 216 + r0 * 8:216 + r1 * 8],
                    sl.rearrange("p (mh wq ws) -> p mh wq ws", wq=8, ws=3),
                    axis=AX.X)

            def sp5_rows_early():
                """map slot 5: row pools for hq blocks within rows 0..15/17."""
                allp_v = allp[:, 5 * S:6 * S]
                nc.vector.reduce_sum(
                    allp_v[:, 1:7],
                    t358[:, 0:48].rearrange(
                        "p (hq hs wq) -> p hq wq hs", hq=2, hs=8), axis=AX.X)
                nc.vector.reduce_sum(
                    allp_v[:, 10:34],
                    t358[:, 72:168].rearrange(
                        "p (hq hs wq) -> p hq wq hs", hq=4, hs=4), axis=AX.X)
                nc.vector.reduce_sum(
                    allp_v[:, 46:94],
                    t358[:, 216:360].rearrange(
                        "p (hq hs wq) -> p hq wq hs", hq=6, hs=3), axis=AX.X)

            def sp5_rows_late():
                """remaining hq blocks + 1x1 grid (sum of the 3x3 grid)."""
                allp_v = allp[:, 5 * S:6 * S]
                nc.vector.reduce_sum(
                    allp_v[:, 7:10],
                    t358[:, 48:72].rearrange(
                        "p (hq hs wq) -> p hq wq hs", hq=1, hs=8), axis=AX.X)
                nc.vector.reduce_sum(
                    allp_v[:, 34:46],
                    t358[:, 168:216].rearrange(
                        "p (hq hs wq) -> p hq wq hs", hq=2, hs=4), axis=AX.X)
                nc.vector.reduce_sum(
                    allp_v[:, 94:110],
                    t358[:, 360:408].rearrange(
                        "p (hq hs wq) -> p hq wq hs", hq=2, hs=3), axis=AX.X)
                nc.vector.reduce_sum(
                    allp_v[:, 0:1],
                    allp_v[:, 1:10].rearrange("p (o v) -> p o v", o=1),
                    axis=AX.X)

            def valn_mul(j):
                nc.gpsimd.tensor_mul(valn[:, j * S:(j + 1) * S],
                                     allp[:, (2 + j) * S:(3 + j) * S],
                                     c_sclv[:, j * S:(j + 1) * S])

            # ---------- primary 1x1 convs (fp8 DR), half-quad grain ----------
            # dsts: (map idx, weight view, m-slice lo, out ch tot, bias col)
            def prim_quad(mi, wv, mlo, mtot, bcol, rq, psA):
                """4 row-blocks rq*4..rq*4+3 -> one 4-bank psum + 1 ACT evac."""
                ps = psA.tile([P, 2048], f32, name="pA")
                for sub in range(4):
                    rb = rq * 4 + sub
                    xt, lo = xv(rb * 384)
                    for kc in range(2):
                        lh = wv[:, kc, :, mlo:mlo + 128] if mtot == 256 \
                            else wv[:, kc]
                        nc.tensor.matmul(
                            ps[:, sub * 512:sub * 512 + 384],
                            lh, xt[:, 2 * kc:2 * kc + 2, lo:lo + 384],
                            start=(kc == 0), stop=(kc == 1),
                            perf_mode=DR)
                dst = mk(maps[mi][:], MPAD + (rq * 16 + 1) * WP + 1,
                         [[4 * WP, 4], [WP, 4], [1, 96]])
                src = mk(ps[:], 0, [[512, 4], [96, 4], [1, 96]])
                nc.scalar.activation(dst, src, AF.Relu,
                                     bias=c_bias[:, bcol:bcol + 1])

            def dw_group(ci, g, psD):
                """2 output row-blocks of depthwise for map ci."""
                bcol = (1, 4, 5)[ci]
                m = maps[ci]
                ps = psD.tile([P, 1024], f32, name="pD", tag="pG")
                for j in range(2):
                    r0 = (g * 2 + j) * 4
                    reg = ps[:, j * 512:j * 512 + 384]
                    for t in range(6):
                        dx = t % 3
                        if t < 3:  # DR pair: dy=-1 and dy=+1
                            rhs = mk(m[:], MPAD + r0 * WP + dx,
                                     [[2 * WP, 2], [WP, 4], [1, 96]])
                            nc.tensor.matmul(reg, dgav[:, ci, dx], rhs,
                                             start=(t == 0), stop=False,
                                             perf_mode=DR)
                        else:      # single: dy=0
                            rhs = mk(m[:], MPAD + (r0 + 1) * WP + dx,
                                     [[WP, 4], [1, 96]])
                            nc.tensor.matmul(reg, dgsv[:, ci, dx], rhs,
                                             start=False, stop=(t == 5))
                bdst = mk(kbm[ci][:], g * 768, [[384, 2], [96, 4], [1, 96]])
                bsrc = mk(ps[:], 0, [[512, 2], [1, 384]])
                nc.scalar.activation(bdst, bsrc, AF.Relu,
                                     bias=c_bias[:, bcol:bcol + 1])

            def chunk_q(n, psG, qsbp):
                """Q conv for 512-px chunk n -> qsb fp8 [p, kq, 512]."""
                xt, lo = xv(n * NCW)
                qsb = qsbp.tile([P, 1024], f8, name="qsb")
                ps = psG.tile([P, 1024], f32, name="pG", tag="pG")
                for kq in range(2):
                    for kc in range(2):
                        nc.tensor.matmul(
                            ps[:, kq * 512:(kq + 1) * 512],
                            wv_q[:, kc, :, kq * 128:(kq + 1) * 128],
                            xt[:, 2 * kc:2 * kc + 2, lo:lo + NCW],
                            start=(kc == 0), stop=(kc == 1), perf_mode=DR)
                for kq in range(2):
                    nc.scalar.activation(
                        qsb[:, kq * 512:(kq + 1) * 512],
                        ps[:, kq * 512:(kq + 1) * 512], AF.Relu,
                        bias=c_bias[:, 6 + kq:7 + kq])
                return qsb

            def sim_pair(m, qs, psG):
                """simT + exp for chunks 2m, 2m+1 -> pe_all slice."""
                sps = psG.tile([P, 1024], f32, name="pG", tag="pG")
                lh = mk(keyn8[:], 0, [[SP, 2], [1, SP]])
                for h in range(2):
                    n = 2 * m + h
                    qsb = qs.pop(n)
                    rhs = mk(qsb[:], 0, [[512, 2], [1, 512]])
                    nc.tensor.matmul(sps[0:SP, h * 512:(h + 1) * 512],
                                     lh, rhs, start=True, stop=True,
                                     perf_mode=DR)
                nc.scalar.activation(pe_all[0:S, 2 * m * NCW:(2 * m + 2) * NCW],
                                     sps[0:S, :], AF.Exp)

            def vt_tp(j, psG, eng="s"):
                tp = psG.tile([P, 1024], f32, name="pG", tag="pG")
                tpv = tp[0:S, 0:64].bitcast(bf16)
                nc.tensor.transpose(tpv, valn[:, j * S:(j + 1) * S],
                                    c_id[:])
                if eng == "s":
                    nc.scalar.copy(vT[0:S, j * 128:(j + 1) * 128], tpv)
                else:
                    nc.vector.tensor_copy(vT[0:S, j * 128:(j + 1) * 128], tpv)

            nev = [0]

            def ctx_cc(m, cv, psG, cbp, eng="v"):
                """ctx matmul + evac + store for (pair m, channel block cv)."""
                T = psG.tile([P, 1024], f32, name="pG", tag="pG")
                for h in range(2):
                    nc.tensor.matmul(
                        T[:, h * 512:(h + 1) * 512],
                        vT[0:S, cv * 128:(cv + 1) * 128],
                        pe_all[0:S, (2 * m + h) * NCW:(2 * m + h + 1) * NCW],
                        start=True, stop=True)
                cb = cbp.tile([P, 1024], bf16, name="cb")
                if eng == "sv":
                    nc.scalar.copy(cb[:, 0:512], T[:, 0:512])
                    nc.vector.tensor_copy(cb[:, 512:1024], T[:, 512:1024])
                elif eng == "s":
                    nc.scalar.copy(cb[:], T[:])
                else:
                    nc.vector.tensor_copy(cb[:], T[:])
                if cv == 3:
                    dq = (nc.gpsimd, nc.scalar, nc.sync)[m % 3]
                else:
                    dq = nc.sync
                dq.dma_start(
                    ctx_r[:, cv:cv + 1, 2 * m * NCW:(2 * m + 2) * NCW],
                    cb[:].rearrange("p (c n) -> p c n", c=1))

            # ================= emission =================
            # Phase 1: k-quads interleaved with dw-k groups (fills the x-DMA
            # pipeline); DVE runs the keyn pool chain chunked alongside.
            with tc.tile_pool(name="psA", bufs=1, space="PSUM") as psA, \
                    tc.tile_pool(name="psD", bufs=2, space="PSUM") as psD:
                kq_ = lambda rq: prim_quad(0, wv_k, 0, 128, 0, rq, psA)
                dg_ = lambda g: dw_group(0, g, psD)
                kq_(0); kq_(1); dg_(0); dg_(1)
                map_pool(0, 0, 0, 6)
                kq_(2); dg_(2); dg_(3)
                kq_(3); dg_(4)
                map_pool(0, 0, 6, 12)
                dg_(5)
                kq_(4); dg_(6); dg_(7)
                map_pool(0, 0, 12, 18)
                kb_rowsum(0, 0, 8)
                kq_(5); dg_(8); dg_(9)
                map_pool(0, 0, 18, 24)
                dg_(10); dg_(11)
                kb_rowsum(0, 8, 16)
                kb_rowsum(0, 16, 24)
                kb_colred(0, 0, 24)
                smallpools(0, 2)
                nc.gpsimd.tensor_mul(kn[:, :, 0:S],
                                     allp[:, 0:2 * S].rearrange(
                                         "p (o s) -> p o s", o=2),
                                     c_sclk[:].rearrange("p (o s) -> p o s", o=2))

            # Phase 2: value primaries; first map1 pool chunks start here,
            # the rest run inside the loop's tick slots.
            with tc.tile_pool(name="psA", bufs=2, space="PSUM") as psA:
                for rq in range(6):
                    prim_quad(1, wv_v, 0, 256, 2, rq, psA)
                map_pool(1, 2, 0, 6)
                map_pool(1, 2, 6, 12)
                for rq in range(6):
                    prim_quad(2, wv_v, 128, 256, 3, rq, psA)

            # ---------- main interleave: dw-v + q/sim + ctx ----------
            with tc.tile_pool(name="psG", bufs=4, space="PSUM") as psG, \
                    tc.tile_pool(name="qsb", bufs=6) as qsbp, \
                    tc.tile_pool(name="ctxb", bufs=6) as cbp:
                NT = 24
                qs = {}
                pending = {}   # tick -> list of thunks

                def defer(t, fn):
                    pending.setdefault(t, []).append(fn)

                CV0_T, CV1_T, CV2_T = 5, 12, 18

                def side_work(t):
                    for fn in pending.pop(t, ()):
                        fn()
                    if t < 18:
                        k = t
                        qs[k] = chunk_q(k, psG, qsbp)
                        if k % 2 == 1:
                            m = k // 2
                            defer(2 * m + 3, lambda m=m: sim_pair(m, qs, psG))
                            defer(max(2 * m + 5, CV0_T),
                                  lambda m=m: ctx_cc(m, 0, psG, cbp))
                            defer(91 if m == 8 else max(2 * m + 6, CV1_T + m),
                                  lambda m=m, e=("sv" if m == 8 else "v"):
                                  ctx_cc(m, 1, psG, cbp, e))
                            defer(92 + m if m >= 7 else max(2 * m + 7, CV2_T + m),
                                  lambda m=m, e=("sv" if m >= 7 else "v"):
                                  ctx_cc(m, 2, psG, cbp, e))

                # DVE side-chain per tick (kept small so qsb evacs stay fresh)
                dve_side = {
                    0: [lambda: map_pool(1, 2, 12, 18)],
                    1: [lambda: map_pool(1, 2, 18, 24)],
                    2: [lambda: smallpools(2, 3), lambda: valn_mul(0)],
                    3: [lambda: map_pool(2, 3, 0, 6)],
                    4: [lambda: map_pool(2, 3, 6, 12), lambda: vt_tp(0, psG)],
                    5: [lambda: map_pool(2, 3, 12, 18),
                        lambda: kb_rowsum(1, 0, 8)],
                    6: [lambda: map_pool(2, 3, 18, 24)],
                    7: [lambda: smallpools(3, 4), lambda: valn_mul(1)],
                    9: [lambda: kb_rowsum(1, 8, 16)],
                    10: [lambda: kb_colred(1, 0, 16)],
                    11: [lambda: vt_tp(1, psG)],
                    12: [lambda: kb_rowsum(1, 16, 24),
                         lambda: kb_colred(1, 16, 24)],
                    13: [lambda: smallpools(4, 5), lambda: valn_mul(2)],
                    15: [lambda: vt_tp(2, psG)],
                    16: [lambda: kb_rowsum(2, 0, 8),
                         lambda: nc.sync.dma_start(pe_d[:, 0:8 * NCW],
                                                   pe_all[0:S, 0:8 * NCW])],
                    22: [lambda: kb_rowsum(2, 16, 20),
                         lambda: kb_colred(2, 16, 20),
                         lambda: sp5_cols(16, 20),
                         lambda: sp5_rows_early()],
                    20: [lambda: kb_rowsum(2, 8, 16),
                         lambda: kb_colred(2, 0, 16)],
                    21: [lambda: sp5_cols(0, 16),
                         lambda: nc.sync.dma_start(
                             pe_d[:, 8 * NCW:16 * NCW],
                             pe_all[0:S, 8 * NCW:16 * NCW])],
                }

                for t in range(NT):
                    ci = 1 if t < 12 else 2
                    dw_group(ci, t % 12, psG)
                    if t == 23:
                        kb_rowsum(2, 20, 24)
                        kb_colred(2, 20, 24)
                        sp5_cols(20, 24)
                        sp5_rows_late()
                        valn_mul(3)
                    side_work(t)
                    for fn in dve_side.get(t, ()):
                        fn()
                # flush held-back cv work as PE filler under the final
                # pool chain, then vT3 and the cv3 stream
                for t in sorted(pending):
                    for fn in pending.pop(t):
                        fn()
                vt_tp(3, psG, "v")
                for m in range(9):
                    ctx_cc(m, 3, psG, cbp, "sv")
                nc.scalar.dma_start(pe_d[:, 16 * NCW:], pe_all[0:S, 16 * NCW:])

    nc.compile()
    return nc


def prep_host_inputs(inputs):
    """Fold BN affine into weights, quantize to fp8, build aux tensors."""
    import ml_dtypes
    F8 = ml_dtypes.float8_e4m3fn
    g = lambda a: np.ascontiguousarray(np.asarray(a, dtype=np.float32))
    wq = (g(inputs["q_g"])[:, None] * g(inputs["q_w"])[:, :, 0, 0]).T
    wkp = (g(inputs["kp_g"])[:, None] * g(inputs["kp_w"])[:, :, 0, 0]).T
    wvp = (g(inputs["vp_g"])[:, None] * g(inputs["vp_w"])[:, :, 0, 0]).T
    wkc = g(inputs["kc_g"])[:, None, None] * g(inputs["kc_w"])[:, 0]
    wvc = g(inputs["vc_g"])[:, None, None] * g(inputs["vc_w"])[:, 0]

    def pack_primary(wt):  # [512, M] -> [p, kc, o, M] flat
        m = wt.shape[1]
        return np.ascontiguousarray(
            wt.reshape(2, 2, 128, m).transpose(2, 0, 1, 3).reshape(
                128, 4 * m)).astype(F8)

    dga = np.zeros((3, 3, 128, 2, 128), np.float32)
    dgs = np.zeros((3, 3, 128, 128), np.float32)
    taps = [wkc, wvc[:128], wvc[128:]]
    for ci in range(3):
        w = taps[ci]
        for dx in range(3):
            dga[ci, dx, :, 0] = np.diag(w[:, 0, dx])
            dga[ci, dx, :, 1] = np.diag(w[:, 2, dx])
            dgs[ci, dx] = np.diag(w[:, 1, dx])

    scale110 = np.zeros(S, np.float32)
    scale110[0] = 1.0 / 9216
    scale110[1:10] = 1.0 / 1024
    scale110[10:46] = 1.0 / 256
    scale110[46:110] = 1.0 / 144
    sclk = np.broadcast_to(np.tile(scale110 / 16.0, 2), (128, 2 * S))
    sclv = np.broadcast_to(np.tile(scale110, 4), (128, 4 * S))

    bias = np.zeros((128, 8), np.float32)
    bias[:, 0] = g(inputs["kp_b"])
    bias[:, 1] = g(inputs["kc_b"])
    bias[:, 2] = g(inputs["vp_b"])[:128]
    bias[:, 3] = g(inputs["vp_b"])[128:]
    bias[:, 4] = g(inputs["vc_b"])[:128]
    bias[:, 5] = g(inputs["vc_b"])[128:]
    bias[:, 6] = g(inputs["q_b"])[:128]
    bias[:, 7] = g(inputs["q_b"])[128:]

    return {
        "wkp8": pack_primary(wkp),
        "wvp8": pack_primary(wvp),
        "wq8": pack_primary(wq),
        "dga": np.ascontiguousarray(
            dga.transpose(2, 0, 1, 3, 4).reshape(128, 2304)).astype(F8),
        "dgs": np.ascontiguousarray(
            dgs.transpose(2, 0, 1, 3).reshape(128, 1152)).astype(F8),
        "ident": np.eye(128, dtype=ml_dtypes.bfloat16),
        "sclk": np.ascontiguousarray(sclk),
        "sclv": np.ascontiguousarray(sclv),
        "bias": bias,
    }


def make_in_maps(inputs):
    import ml_dtypes
    host = prep_host_inputs(inputs)
    x = np.asarray(inputs["x"], dtype=np.float32)
    B = x.shape[0]
    in_maps = []
    for b in range(B):
        m = dict(host)
        m["x8"] = np.ascontiguousarray(x[b].reshape(512, HW)).astype(
            ml_dtypes.float8_e4m3fn)
        in_maps.append(m)
    return in_maps


_NC = None


def get_nc():
    global _NC
    if _NC is None:
        _NC = build_bass()
    return _NC


def kernel(**inputs):
    from concourse import bass_utils
    nc = get_nc()
    in_maps = make_in_maps(inputs)
    res = bass_utils.run_bass_kernel_spmd(
        nc, in_maps, core_ids=list(range(len(in_maps))), trace=False)
    x = np.asarray(inputs["x"], dtype=np.float32)
    outs = []
    for b, r in enumerate(res.results):
        ctx = r["ctx"].astype(np.float32)
        den = r["pe"].astype(np.float32).sum(axis=0, keepdims=True)
        outs.append(x[b] + (ctx / den).reshape(512, HH, HH))
    return np.stack(outs, axis=0).astype(np.float32)


# revision 40
# speedup vs baseline: 1.0581x; 1.0581x over previous
"""CAPAttentionModule Trainium2 kernel, v3 (fp8 DoubleRow, deep interleave).

Data-parallel over batch: 8 images -> 8 NeuronCores, one image per core.
Per core (x: [512, 9216] = [C, H*W], H=W=96), fp8 e4m3 on the PE with
DoubleRow (2x contraction packing):
  k1 = relu(Wkp x + b)            [128, HW]
  v1 = relu(Wvp x + b)            [256, HW]
  k2/v2 = relu(dw3x3(.) + b)      depthwise via diagonal fp8 matmuls:
                                  3 DR pair-matmuls (dy=-1/+1) + 3 singles
  key/value pools: 5D strided DVE reduces -> 24x24 sums -> 1/3/6/8 grids
  q  = relu(Wq x + b)             [256, HW]
  simT = keyn^T q; pe = exp(simT) (keyn pre-scaled by pool-norm/16)
  ctx_u = value @ pe              UNNORMALIZED
Device ships ctx_u (bf16) + pe (bf16); host computes
  y = x + ctx_u / pe.sum(tokens).

Schedule: PE order = k-primary, vlo-primary, dw-k, vhi-primary, then a
48-tick interleave of dw-v row-blocks with q/sim and per-chunk ctx
matmuls (cv streams unblock as value pools finish); ctx stores stream
over sync+scalar DMA queues throughout the second half.
"""

import numpy as np

P = 128
HH = 96
WP = 104          # padded map row pitch (fp8); 2*WP % 16 == 0 for DR pairs
HROWS = 98
MPAD = 16         # guard pad before/after map data (junk reads land here)
MAPN = MPAD + HROWS * WP + MPAD
HW = 9216
S = 110
SP = 112          # padded key free pitch (fp8 DR lhsT pair step % 16 == 0)
NCH = 18
NCW = 512
HALF = HW // 2


def build_bass():
    import concourse.bacc as bacc
    import concourse.tile as tile
    from concourse import mybir
    from concourse.bass import AP
    from contextlib import ExitStack

    f32 = mybir.dt.float32
    bf16 = mybir.dt.bfloat16
    f8 = mybir.dt.float8e4
    DR = mybir.MatmulPerfMode.DoubleRow
    AF = mybir.ActivationFunctionType
    AX = mybir.AxisListType

    nc = bacc.Bacc("TRN2", target_bir_lowering=False, debug=False,
                   enable_asserts=False, num_devices=8)

    x8_d = nc.dram_tensor("x8", [512, HW], f8, kind="ExternalInput").ap()
    wkp_d = nc.dram_tensor("wkp8", [128, 512], f8, kind="ExternalInput").ap()
    wvp_d = nc.dram_tensor("wvp8", [128, 1024], f8, kind="ExternalInput").ap()
    wq_d = nc.dram_tensor("wq8", [128, 1024], f8, kind="ExternalInput").ap()
    dga_d = nc.dram_tensor("dga", [128, 2304], f8, kind="ExternalInput").ap()
    dgs_d = nc.dram_tensor("dgs", [128, 1152], f8, kind="ExternalInput").ap()
    id_d = nc.dram_tensor("ident", [128, 128], bf16, kind="ExternalInput").ap()
    sclk_d = nc.dram_tensor("sclk", [128, 2 * S], f32, kind="ExternalInput").ap()
    sclv_d = nc.dram_tensor("sclv", [128, 4 * S], f32, kind="ExternalInput").ap()
    bias_d = nc.dram_tensor("bias", [128, 8], f32, kind="ExternalInput").ap()
    ctx_d = nc.dram_tensor("ctx", [512, HW], bf16, kind="ExternalOutput").ap()
    pe_d = nc.dram_tensor("pe", [S, HW], bf16, kind="ExternalOutput").ap()

    x8_r = x8_d.rearrange("(t p) n -> p t n", p=P)       # [128, 4, 9216]
    ctx_r = ctx_d.rearrange("(cv p) n -> p cv n", p=P)   # [128, 4, 9216]

    def mk(base_ap, off, dims):
        """Manual AP: keep base's partition entry, custom free dims."""
        return AP(base_ap.tensor, base_ap.offset + off,
                  [list(base_ap.ap[0])] + [[s, n] for s, n in dims])

    with tile.TileContext(nc) as tc:
        with ExitStack() as top:
            cpool = top.enter_context(tc.tile_pool(name="consts", bufs=1))
            kpool = top.enter_context(tc.tile_pool(name="keep", bufs=1))
            tmpp = top.enter_context(tc.tile_pool(name="tmp", bufs=1))

            x8t = [kpool.tile([P, 4 * 1536], f8, name=f"x8_{i6}")
                   for i6 in range(6)]
            nc.sync.dma_start(x8t[0][:].rearrange("p (t n) -> p t n", t=4),
                              x8_r[:, :, 0:1536])
            c_wkp = cpool.tile([P, 512], f8)
            nc.scalar.dma_start(c_wkp[:], wkp_d)
            c_bias = cpool.tile([P, 8], f32)
            nc.scalar.dma_start(c_bias[:], bias_d)
            c_dga = cpool.tile([P, 2304], f8)
            c_dgs = cpool.tile([P, 1152], f8)
            c_wvp = cpool.tile([P, 1024], f8)
            c_wq = cpool.tile([P, 1024], f8)
            c_id = cpool.tile([P, 128], bf16)
            c_sclk = cpool.tile([P, 2 * S], f32)
            c_sclv = cpool.tile([P, 4 * S], f32)
            # prewarm ACT function tables while DMAs land
            warm = cpool.tile([P, 2], f32)
            nc.scalar.activation(warm[:, 0:1], c_bias[:, 0:1], AF.Relu)
            nc.scalar.activation(warm[:, 1:2], c_bias[:, 0:1], AF.Exp)

            maps = [kpool.tile([P, MAPN], f8, name=f"map{i}") for i in range(3)]
            kbm = [kpool.tile([P, HW], bf16, name=f"kb{i}") for i in range(3)]
            rs = kpool.tile([P, 2304], bf16)
            p24 = kpool.tile([P, 6 * 576], f32)
            allp = kpool.tile([P, 6 * S], f32)
            keyn8 = kpool.tile([P, 2 * SP], f8)     # [kq, 112] padded
            valn = kpool.tile([P, 4 * S], bf16)
            vT = kpool.tile([P, 512], bf16)
            pe_all = kpool.tile([P, NCH * NCW], bf16)
            t358 = kpool.tile([P, 408], f32)

            # zero map borders: row 0, row 97, col 0, col 97
            for m in maps:
                mv = mk(m[:], MPAD, [[WP, HROWS], [1, WP]])
                nc.gpsimd.memset(mv[:, 0:1, :], 0.0)
                nc.gpsimd.memset(mv[:, 97:98, :], 0.0)
                nc.gpsimd.memset(mv[:, 1:97, 0:1], 0.0)
                nc.gpsimd.memset(mv[:, 1:97, 97:98], 0.0)
            # zero keyn pad columns (110, 111 of each kq)
            kn = keyn8[:].rearrange("p (o s) -> p o s", o=2)
            nc.gpsimd.memset(kn[:, :, S:SP], 0.0)
            nc.scalar.dma_start(c_dga[:], dga_d)
            nc.scalar.dma_start(c_dgs[:], dgs_d)
            for i6 in range(1, 6):
                nc.sync.dma_start(
                    x8t[i6][:].rearrange("p (t n) -> p t n", t=4),
                    x8_r[:, :, i6 * 1536:(i6 + 1) * 1536])
            nc.scalar.dma_start(c_wvp[:], wvp_d)
            nc.scalar.dma_start(c_wq[:], wq_d)
            nc.scalar.dma_start(c_id[:], id_d)
            nc.scalar.dma_start(c_sclk[:], sclk_d)
            nc.scalar.dma_start(c_sclv[:], sclv_d)

            def xv(lo):
                """(tile, local offset) for pixel range starting at lo."""
                t = x8t[lo // 1536]
                return t[:].rearrange("p (t n) -> p t n", t=4), lo % 1536

            wv_k = c_wkp[:].rearrange("p (kc o m) -> p kc o m", kc=2, o=2)
            wv_v = c_wvp[:].rearrange("p (kc o m) -> p kc o m", kc=2, o=2)
            wv_q = c_wq[:].rearrange("p (kc o m) -> p kc o m", kc=2, o=2)
            dgav = c_dga[:].rearrange("p (ci dx o m) -> p ci dx o m", ci=3, dx=3, o=2)
            dgsv = c_dgs[:].rearrange("p (ci dx m) -> p ci dx m", ci=3, dx=3)

            # stage-2 pools: maps [m0, m1) of p24 -> allp (4 grids each)
            def smallpools(m0, m1):
                m = m1 - m0
                allp_v = allp[:, m0 * S:m1 * S].rearrange("p (m s) -> p m s", s=S)
                p24s = p24[:, m0 * 576:m1 * 576]
                nc.vector.reduce_sum(
                    allp_v[:, :, 0:1],
                    p24s.rearrange("p (m s) -> p m s", s=576), axis=AX.X)
                tmp = tmpp.tile([P, 1152], f32, name="tmp", tag="tmp")
                nc.vector.reduce_sum(
                    tmp[:, 0:m * 72],
                    p24s.rearrange("p (mh wq ws) -> p mh wq ws", wq=3, ws=8),
                    axis=AX.X)
                nc.vector.reduce_sum(
                    allp_v[:, :, 1:10],
                    tmp[:, 0:m * 72].rearrange(
                        "p (m hq hs wq) -> p m hq wq hs", m=m, hq=3, hs=8),
                    axis=AX.X)
                tmp6 = tmpp.tile([P, 1152], f32, name="tmp6", tag="tmp")
                nc.vector.reduce_sum(
                    tmp6[:, 0:m * 144],
                    p24s.rearrange("p (mh wq ws) -> p mh wq ws", wq=6, ws=4),
                    axis=AX.X)
                nc.vector.reduce_sum(
                    allp_v[:, :, 10:46],
                    tmp6[:, 0:m * 144].rearrange(
                        "p (m hq hs wq) -> p m hq wq hs", m=m, hq=6, hs=4),
                    axis=AX.X)
                tmp8 = tmpp.tile([P, 1152], f32, name="tmp8", tag="tmp")
                nc.vector.reduce_sum(
                    tmp8[:, 0:m * 192],
                    p24s.rearrange("p (mh wq ws) -> p mh wq ws", wq=8, ws=3),
                    axis=AX.X)
                nc.vector.reduce_sum(
                    allp_v[:, :, 46:110],
                    tmp8[:, 0:m * 192].rearrange(
                        "p (m hq hs wq) -> p m hq wq hs", m=m, hq=8, hs=3),
                    axis=AX.X)

            def map_pool(mi, slot, r0, r1):
                """4x4 stage-1 pool of map mi, stage-1 rows [r0, r1)."""
                src = mk(maps[mi][:], MPAD + (4 * r0 + 1) * WP + 1,
                         [[4 * WP, r1 - r0], [4, 24], [WP, 4], [1, 4]])
                nc.vector.reduce_sum(
                    p24[:, slot * 576 + r0 * 24:slot * 576 + r1 * 24],
                    src, axis=AX.XY)

            def kb_rowsum(ci, r0, r1):
                """dw-map ci: sum 4 map-rows of each stage-1 row r0..r1 -> rs
                (bf16 all-SBUF adds run in the DVE 2x mode)."""
                n = r1 - r0
                dst = mk(rs[:], r0 * 96, [[1, n * 96]])
                a = [mk(kbm[ci][:], r0 * 384 + i * 96, [[384, n], [1, 96]])
                     for i in range(4)]
                nc.vector.tensor_tensor(dst, a[0], a[1],
                                        op=mybir.AluOpType.add)
                nc.vector.tensor_tensor(dst, dst, a[2],
                                        op=mybir.AluOpType.add)
                nc.vector.tensor_tensor(dst, dst, a[3],
                                        op=mybir.AluOpType.add)

            def kb_colred(ci, r0, r1):
                """col-reduce rs rows r0..r1 into p24 slot for dw map ci."""
                slot = (1, 4, 5)[ci]
                src = mk(rs[:], r0 * 96, [[96, r1 - r0], [4, 24], [1, 4]])
                nc.vector.reduce_sum(
                    p24[:, slot * 576 + r0 * 24:slot * 576 + r1 * 24],
                    src, axis=AX.X)

            def sp5_cols(r0, r1):
                """map slot 5: per-row column pools for grids 3/6/8."""
                sl = p24[:, 5 * 576 + r0 * 24:5 * 576 + r1 * 24]
                nc.vector.reduce_sum(
                    t358[:, r0 * 3:r1 * 3],
                    sl.rearrange("p (mh wq ws) -> p mh wq ws", wq=3, ws=8),
                    axis=AX.X)
                nc.vector.reduce_sum(
                    t358[:, 72 + r0 * 6:72 + r1 * 6],
                    sl.rearrange("p (mh wq ws) -> p mh wq ws", wq=6, ws=4),
                    axis=AX.X)
                nc.vector.reduce_sum(
                    t358[:, 216 + r0 * 8:216 + r1 * 8],
                    sl.rearrange("p (mh wq ws) -> p mh wq ws", wq=8, ws=3),
                    axis=AX.X)

            def sp5_rows_early():
                """map slot 5: row pools for hq blocks within rows 0..15/17."""
                allp_v = allp[:, 5 * S:6 * S]
                nc.vector.reduce_sum(
                    allp_v[:, 1:7],
                    t358[:, 0:48].rearrange(
                        "p (hq hs wq) -> p hq wq hs", hq=2, hs=8), axis=AX.X)
                nc.vector.reduce_sum(
                    allp_v[:, 10:34],
                    t358[:, 72:168].rearrange(
                        "p (hq hs wq) -> p hq wq hs", hq=4, hs=4), axis=AX.X)
                nc.vector.reduce_sum(
                    allp_v[:, 46:94],
                    t358[:, 216:360].rearrange(
                        "p (hq hs wq) -> p hq wq hs", hq=6, hs=3), axis=AX.X)

            def sp5_rows_late():
                """remaining hq blocks + 1x1 grid (sum of the 3x3 grid)."""
                allp_v = allp[:, 5 * S:6 * S]
                nc.vector.reduce_sum(
                    allp_v[:, 7:10],
                    t358[:, 48:72].rearrange(
                        "p (hq hs wq) -> p hq wq hs", hq=1, hs=8), axis=AX.X)
                nc.vector.reduce_sum(
                    allp_v[:, 34:46],
                    t358[:, 168:216].rearrange(
                        "p (hq hs wq) -> p hq wq hs", hq=2, hs=4), axis=AX.X)
                nc.vector.reduce_sum(
                    allp_v[:, 94:110],
                    t358[:, 360:408].rearrange(
                        "p (hq hs wq) -> p hq wq hs", hq=2, hs=3), axis=AX.X)
                nc.vector.reduce_sum(
                    allp_v[:, 0:1],
                    allp_v[:, 1:10].rearrange("p (o v) -> p o v", o=1),
                    axis=AX.X)

            def valn_mul(j):
                nc.gpsimd.tensor_mul(valn[:, j * S:(j + 1) * S],
                                     allp[:, (2 + j) * S:(3 + j) * S],
                                     c_sclv[:, j * S:(j + 1) * S])

            # ---------- primary 1x1 convs (fp8 DR), half-quad grain ----------
            # dsts: (map idx, weight view, m-slice lo, out ch tot, bias col)
            def prim_quad(mi, wv, mlo, mtot, bcol, rq, psA):
                """4 row-blocks rq*4..rq*4+3 -> one 4-bank psum + 1 ACT evac."""
                ps = psA.tile([P, 2048], f32, name="pA")
                for sub in range(4):
                    rb = rq * 4 + sub
                    xt, lo = xv(rb * 384)
                    for kc in range(2):
                        lh = wv[:, kc, :, mlo:mlo + 128] if mtot == 256 \
                            else wv[:, kc]
                        nc.tensor.matmul(
                            ps[:, sub * 512:sub * 512 + 384],
                            lh, xt[:, 2 * kc:2 * kc + 2, lo:lo + 384],
                            start=(kc == 0), stop=(kc == 1),
                            perf_mode=DR)
                dst = mk(maps[mi][:], MPAD + (rq * 16 + 1) * WP + 1,
                         [[4 * WP, 4], [WP, 4], [1, 96]])
                src = mk(ps[:], 0, [[512, 4], [96, 4], [1, 96]])
                nc.scalar.activation(dst, src, AF.Relu,
                                     bias=c_bias[:, bcol:bcol + 1])

            def dw_group(ci, g, psD):
                """2 output row-blocks of depthwise for map ci."""
                bcol = (1, 4, 5)[ci]
                m = maps[ci]
                ps = psD.tile([P, 1024], f32, name="pD", tag="pG")
                for j in range(2):
                    r0 = (g * 2 + j) * 4
                    reg = ps[:, j * 512:j * 512 + 384]
                    for t in range(6):
                        dx = t % 3
                        if t < 3:  # DR pair: dy=-1 and dy=+1
                            rhs = mk(m[:], MPAD + r0 * WP + dx,
                                     [[2 * WP, 2], [WP, 4], [1, 96]])
                            nc.tensor.matmul(reg, dgav[:, ci, dx], rhs,
                                             start=(t == 0), stop=False,
                                             perf_mode=DR)
                        else:      # single: dy=0
                            rhs = mk(m[:], MPAD + (r0 + 1) * WP + dx,
                                     [[WP, 4], [1, 96]])
                            nc.tensor.matmul(reg, dgsv[:, ci, dx], rhs,
                                             start=False, stop=(t == 5))
                bdst = mk(kbm[ci][:], g * 768, [[384, 2], [96, 4], [1, 96]])
                bsrc = mk(ps[:], 0, [[512, 2], [1, 384]])
                nc.scalar.activation(bdst, bsrc, AF.Relu,
                                     bias=c_bias[:, bcol:bcol + 1])

            def chunk_q(n, psG, qsbp):
                """Q conv for 512-px chunk n -> qsb fp8 [p, kq, 512]."""
                xt, lo = xv(n * NCW)
                qsb = qsbp.tile([P, 1024], f8, name="qsb")
                ps = psG.tile([P, 1024], f32, name="pG", tag="pG")
                for kq in range(2):
                    for kc in range(2):
                        nc.tensor.matmul(
                            ps[:, kq * 512:(kq + 1) * 512],
                            wv_q[:, kc, :, kq * 128:(kq + 1) * 128],
                            xt[:, 2 * kc:2 * kc + 2, lo:lo + NCW],
                            start=(kc == 0), stop=(kc == 1), perf_mode=DR)
                for kq in range(2):
                    nc.scalar.activation(
                        qsb[:, kq * 512:(kq + 1) * 512],
                        ps[:, kq * 512:(kq + 1) * 512], AF.Relu,
                        bias=c_bias[:, 6 + kq:7 + kq])
                return qsb

            def sim_pair(m, qs, psG):
                """simT + exp for chunks 2m, 2m+1 -> pe_all slice."""
                sps = psG.tile([P, 1024], f32, name="pG", tag="pG")
                lh = mk(keyn8[:], 0, [[SP, 2], [1, SP]])
                for h in range(2):
                    n = 2 * m + h
                    qsb = qs.pop(n)
                    rhs = mk(qsb[:], 0, [[512, 2], [1, 512]])
                    nc.tensor.matmul(sps[0:SP, h * 512:(h + 1) * 512],
                                     lh, rhs, start=True, stop=True,
                                     perf_mode=DR)
                nc.scalar.activation(pe_all[0:S, 2 * m * NCW:(2 * m + 2) * NCW],
                                     sps[0:S, :], AF.Exp)

            def vt_tp(j, psG, eng="s"):
                tp = psG.tile([P, 1024], f32, name="pG", tag="pG")
                tpv = tp[0:S, 0:64].bitcast(bf16)
                nc.tensor.transpose(tpv, valn[:, j * S:(j + 1) * S],
                                    c_id[:])
                if eng == "s":
                    nc.scalar.copy(vT[0:S, j * 128:(j + 1) * 128], tpv)
                else:
                    nc.vector.tensor_copy(vT[0:S, j * 128:(j + 1) * 128], tpv)

            nev = [0]

            def ctx_cc(m, cv, psG, cbp, eng="v"):
                """ctx matmul + evac + store for (pair m, channel block cv)."""
                T = psG.tile([P, 1024], f32, name="pG", tag="pG")
                for h in range(2):
                    nc.tensor.matmul(
                        T[:, h * 512:(h + 1) * 512],
                        vT[0:S, cv * 128:(cv + 1) * 128],
                        pe_all[0:S, (2 * m + h) * NCW:(2 * m + h + 1) * NCW],
                        start=True, stop=True)
                cb = cbp.tile([P, 1024], bf16, name="cb")
                if eng == "s":
                    nc.scalar.copy(cb[:], T[:])
                else:
                    nc.vector.tensor_copy(cb[:], T[:])
                if cv == 3:
                    dq = (nc.gpsimd, nc.scalar, nc.sync)[m % 3]
                else:
                    dq = nc.sync
                dq.dma_start(
                    ctx_r[:, cv:cv + 1, 2 * m * NCW:(2 * m + 2) * NCW],
                    cb[:].rearrange("p (c n) -> p c n", c=1))

            # ================= emission =================
            # Phase 1: k-quads interleaved with dw-k groups (fills the x-DMA
            # pipeline); DVE runs the keyn pool chain chunked alongside.
            with tc.tile_pool(name="psA", bufs=1, space="PSUM") as psA, \
                    tc.tile_pool(name="psD", bufs=2, space="PSUM") as psD:
                kq_ = lambda rq: prim_quad(0, wv_k, 0, 128, 0, rq, psA)
                dg_ = lambda g: dw_group(0, g, psD)
                kq_(0); kq_(1); dg_(0); dg_(1)
                map_pool(0, 0, 0, 6)
                kq_(2); dg_(2); dg_(3)
                kq_(3); dg_(4)
                map_pool(0, 0, 6, 12)
                dg_(5)
                kq_(4); dg_(6); dg_(7)
                map_pool(0, 0, 12, 18)
                kb_rowsum(0, 0, 8)
                kq_(5); dg_(8); dg_(9)
                map_pool(0, 0, 18, 24)
                dg_(10); dg_(11)
                kb_rowsum(0, 8, 16)
                kb_rowsum(0, 16, 24)
                kb_colred(0, 0, 24)
                smallpools(0, 2)
                nc.gpsimd.tensor_mul(kn[:, :, 0:S],
                                     allp[:, 0:2 * S].rearrange(
                                         "p (o s) -> p o s", o=2),
                                     c_sclk[:].rearrange("p (o s) -> p o s", o=2))

            # Phase 2: value primaries; first map1 pool chunks start here,
            # the rest run inside the loop's tick slots.
            with tc.tile_pool(name="psA", bufs=2, space="PSUM") as psA:
                for rq in range(6):
                    prim_quad(1, wv_v, 0, 256, 2, rq, psA)
                map_pool(1, 2, 0, 6)
                map_pool(1, 2, 6, 12)
                for rq in range(6):
                    prim_quad(2, wv_v, 128, 256, 3, rq, psA)

            # ---------- main interleave: dw-v + q/sim + ctx ----------
            with tc.tile_pool(name="psG", bufs=4, space="PSUM") as psG, \
                    tc.tile_pool(name="qsb", bufs=6) as qsbp, \
                    tc.tile_pool(name="ctxb", bufs=6) as cbp:
                NT = 24
                qs = {}
                pending = {}   # tick -> list of thunks

                def defer(t, fn):
                    pending.setdefault(t, []).append(fn)

                CV0_T, CV1_T, CV2_T = 7, 12, 18

                def side_work(t):
                    for fn in pending.pop(t, ()):
                        fn()
                    if t < 18:
                        k = t
                        qs[k] = chunk_q(k, psG, qsbp)
                        if k % 2 == 1:
                            m = k // 2
                            defer(2 * m + 3, lambda m=m: sim_pair(m, qs, psG))
                            defer(max(2 * m + 5, CV0_T),
                                  lambda m=m: ctx_cc(m, 0, psG, cbp))
                            defer(91 if m == 8 else max(2 * m + 6, CV1_T + m),
                                  lambda m=m, e=("s" if m == 8 else "v"):
                                  ctx_cc(m, 1, psG, cbp, e))
                            defer(92 + m if m >= 7 else max(2 * m + 7, CV2_T + m),
                                  lambda m=m, e=("s" if m % 2 else "v"):
                                  ctx_cc(m, 2, psG, cbp, e))

                # DVE side-chain per tick (kept small so qsb evacs stay fresh)
                dve_side = {
                    0: [lambda: map_pool(1, 2, 12, 18)],
                    1: [lambda: map_pool(1, 2, 18, 24)],
                    2: [lambda: smallpools(2, 3), lambda: valn_mul(0)],
                    3: [lambda: map_pool(2, 3, 0, 6)],
                    4: [lambda: map_pool(2, 3, 6, 12)],
                    5: [lambda: map_pool(2, 3, 12, 18),
                        lambda: kb_rowsum(1, 0, 8)],
                    6: [lambda: map_pool(2, 3, 18, 24),
                        lambda: vt_tp(0, psG)],
                    7: [lambda: smallpools(3, 4), lambda: valn_mul(1)],
                    9: [lambda: kb_rowsum(1, 8, 16)],
                    10: [lambda: kb_colred(1, 0, 16)],
                    11: [lambda: vt_tp(1, psG)],
                    12: [lambda: kb_rowsum(1, 16, 24),
                         lambda: kb_colred(1, 16, 24)],
                    13: [lambda: smallpools(4, 5), lambda: valn_mul(2)],
                    15: [lambda: vt_tp(2, psG)],
                    16: [lambda: kb_rowsum(2, 0, 8),
                         lambda: nc.sync.dma_start(pe_d[:, 0:8 * NCW],
                                                   pe_all[0:S, 0:8 * NCW])],
                    22: [lambda: kb_rowsum(2, 16, 20),
                         lambda: kb_colred(2, 16, 20),
                         lambda: sp5_cols(16, 20),
                         lambda: sp5_rows_early()],
                    20: [lambda: kb_rowsum(2, 8, 16),
                         lambda: kb_colred(2, 0, 16)],
                    21: [lambda: sp5_cols(0, 16),
                         lambda: nc.sync.dma_start(
                             pe_d[:, 8 * NCW:16 * NCW],
                             pe_all[0:S, 8 * NCW:16 * NCW])],
                }

                for t in range(NT):
                    ci = 1 if t < 12 else 2
                    dw_group(ci, t % 12, psG)
                    if t == 23:
                        kb_rowsum(2, 20, 24)
                        kb_colred(2, 20, 24)
                        sp5_cols(20, 24)
                        sp5_rows_late()
                        valn_mul(3)
                    side_work(t)
                    for fn in dve_side.get(t, ()):
                        fn()
                # flush held-back cv work as PE filler under the final
                # pool chain, then vT3 and the cv3 stream
                for t in sorted(pending):
                    for fn in pending.pop(t):
                        fn()
                vt_tp(3, psG, "v")
                for m in range(9):
                    ctx_cc(m, 3, psG, cbp, "s" if m % 2 else "v")
                nc.scalar.dma_start(pe_d[:, 16 * NCW:], pe_all[0:S, 16 * NCW:])

    nc.compile()
    return nc


def prep_host_inputs(inputs):
    """Fold BN affine into weights, quantize to fp8, build aux tensors."""
    import ml_dtypes
    F8 = ml_dtypes.float8_e4m3fn
    g = lambda a: np.ascontiguousarray(np.asarray(a, dtype=np.float32))
    wq = (g(inputs["q_g"])[:, None] * g(inputs["q_w"])[:, :, 0, 0]).T
    wkp = (g(inputs["kp_g"])[:, None] * g(inputs["kp_w"])[:, :, 0, 0]).T
    wvp = (g(inputs["vp_g"])[:, None] * g(inputs["vp_w"])[:, :, 0, 0]).T
    wkc = g(inputs["kc_g"])[:, None, None] * g(inputs["kc_w"])[:, 0]
    wvc = g(inputs["vc_g"])[:, None, None] * g(inputs["vc_w"])[:, 0]

    def pack_primary(wt):  # [512, M] -> [p, kc, o, M] flat
        m = wt.shape[1]
        return np.ascontiguousarray(
            wt.reshape(2, 2, 128, m).transpose(2, 0, 1, 3).reshape(
                128, 4 * m)).astype(F8)

    dga = np.zeros((3, 3, 128, 2, 128), np.float32)
    dgs = np.zeros((3, 3, 128, 128), np.float32)
    taps = [wkc, wvc[:128], wvc[128:]]
    for ci in range(3):
        w = taps[ci]
        for dx in range(3):
            dga[ci, dx, :, 0] = np.diag(w[:, 0, dx])
            dga[ci, dx, :, 1] = np.diag(w[:, 2, dx])
            dgs[ci, dx] = np.diag(w[:, 1, dx])

    scale110 = np.zeros(S, np.float32)
    scale110[0] = 1.0 / 9216
    scale110[1:10] = 1.0 / 1024
    scale110[10:46] = 1.0 / 256
    scale110[46:110] = 1.0 / 144
    sclk = np.broadcast_to(np.tile(scale110 / 16.0, 2), (128, 2 * S))
    sclv = np.broadcast_to(np.tile(scale110, 4), (128, 4 * S))

    bias = np.zeros((128, 8), np.float32)
    bias[:, 0] = g(inputs["kp_b"])
    bias[:, 1] = g(inputs["kc_b"])
    bias[:, 2] = g(inputs["vp_b"])[:128]
    bias[:, 3] = g(inputs["vp_b"])[128:]
    bias[:, 4] = g(inputs["vc_b"])[:128]
    bias[:, 5] = g(inputs["vc_b"])[128:]
    bias[:, 6] = g(inputs["q_b"])[:128]
    bias[:, 7] = g(inputs["q_b"])[128:]

    return {
        "wkp8": pack_primary(wkp),
        "wvp8": pack_primary(wvp),
        "wq8": pack_primary(wq),
        "dga": np.ascontiguousarray(
            dga.transpose(2, 0, 1, 3, 4).reshape(128, 2304)).astype(F8),
        "dgs": np.ascontiguousarray(
            dgs.transpose(2, 0, 1, 3).reshape(128, 1152)).astype(F8),
        "ident": np.eye(128, dtype=ml_dtypes.bfloat16),
        "sclk": np.ascontiguousarray(sclk),
        "sclv": np.ascontiguousarray(sclv),
        "bias": bias,
    }


def make_in_maps(inputs):
    import ml_dtypes
    host = prep_host_inputs(inputs)
    x = np.asarray(inputs["x"], dtype=np.float32)
    B = x.shape[0]
    in_maps = []
    for b in range(B):
        m = dict(host)
        m["x8"] = np.ascontiguousarray(x[b].reshape(512, HW)).astype(
            ml_dtypes.float8_e4m3fn)
        in_maps.append(m)
    return in_maps


_NC = None


def get_nc():
    global _NC
    if _NC is None:
        _NC = build_bass()
    return _NC


def kernel(**inputs):
    from concourse import bass_utils
    nc = get_nc()
    in_maps = make_in_maps(inputs)
    res = bass_utils.run_bass_kernel_spmd(
        nc, in_maps, core_ids=list(range(len(in_maps))), trace=False)
    x = np.asarray(inputs["x"], dtype=np.float32)
    outs = []
    for b, r in enumerate(res.results):
        ctx = r["ctx"].astype(np.float32)
        den = r["pe"].astype(np.float32).sum(axis=0, keepdims=True)
        outs.append(x[b] + (ctx / den).reshape(512, HH, HH))
    return np.stack(outs, axis=0).astype(np.float32)


# revision 41
# speedup vs baseline: 1.0724x; 1.0136x over previous
"""CAPAttentionModule Trainium2 kernel, v3 (fp8 DoubleRow, deep interleave).

Data-parallel over batch: 8 images -> 8 NeuronCores, one image per core.
Per core (x: [512, 9216] = [C, H*W], H=W=96), fp8 e4m3 on the PE with
DoubleRow (2x contraction packing):
  k1 = relu(Wkp x + b)            [128, HW]
  v1 = relu(Wvp x + b)            [256, HW]
  k2/v2 = relu(dw3x3(.) + b)      depthwise via diagonal fp8 matmuls:
                                  3 DR pair-matmuls (dy=-1/+1) + 3 singles
  key/value pools: 5D strided DVE reduces -> 24x24 sums -> 1/3/6/8 grids
  q  = relu(Wq x + b)             [256, HW]
  simT = keyn^T q; pe = exp(simT) (keyn pre-scaled by pool-norm/16)
  ctx_u = value @ pe              UNNORMALIZED
Device ships ctx_u (bf16) + pe (bf16); host computes
  y = x + ctx_u / pe.sum(tokens).

Schedule: PE order = k-primary, vlo-primary, dw-k, vhi-primary, then a
48-tick interleave of dw-v row-blocks with q/sim and per-chunk ctx
matmuls (cv streams unblock as value pools finish); ctx stores stream
over sync+scalar DMA queues throughout the second half.
"""

import numpy as np

P = 128
HH = 96
WP = 104          # padded map row pitch (fp8); 2*WP % 16 == 0 for DR pairs
HROWS = 98
MPAD = 16         # guard pad before/after map data (junk reads land here)
MAPN = MPAD + HROWS * WP + MPAD
HW = 9216
S = 110
SP = 112          # padded key free pitch (fp8 DR lhsT pair step % 16 == 0)
NCH = 18
NCW = 512
HALF = HW // 2


def build_bass():
    import concourse.bacc as bacc
    import concourse.tile as tile
    from concourse import mybir
    from concourse.bass import AP
    from contextlib import ExitStack

    f32 = mybir.dt.float32
    bf16 = mybir.dt.bfloat16
    f8 = mybir.dt.float8e4
    DR = mybir.MatmulPerfMode.DoubleRow
    AF = mybir.ActivationFunctionType
    AX = mybir.AxisListType

    nc = bacc.Bacc("TRN2", target_bir_lowering=False, debug=False,
                   enable_asserts=False, num_devices=8)

    x8_d = nc.dram_tensor("x8", [512, HW], f8, kind="ExternalInput").ap()
    wkp_d = nc.dram_tensor("wkp8", [128, 512], f8, kind="ExternalInput").ap()
    wvp_d = nc.dram_tensor("wvp8", [128, 1024], f8, kind="ExternalInput").ap()
    wq_d = nc.dram_tensor("wq8", [128, 1024], f8, kind="ExternalInput").ap()
    dga_d = nc.dram_tensor("dga", [128, 2304], f8, kind="ExternalInput").ap()
    dgs_d = nc.dram_tensor("dgs", [128, 1152], f8, kind="ExternalInput").ap()
    id_d = nc.dram_tensor("ident", [128, 128], bf16, kind="ExternalInput").ap()
    sclk_d = nc.dram_tensor("sclk", [128, 2 * S], f32, kind="ExternalInput").ap()
    sclv_d = nc.dram_tensor("sclv", [128, 4 * S], f32, kind="ExternalInput").ap()
    bias_d = nc.dram_tensor("bias", [128, 8], f32, kind="ExternalInput").ap()
    ctx_d = nc.dram_tensor("ctx", [512, HW], bf16, kind="ExternalOutput").ap()
    pe_d = nc.dram_tensor("pe", [S, HW], bf16, kind="ExternalOutput").ap()

    x8_r = x8_d.rearrange("(t p) n -> p t n", p=P)       # [128, 4, 9216]
    ctx_r = ctx_d.rearrange("(cv p) n -> p cv n", p=P)   # [128, 4, 9216]

    def mk(base_ap, off, dims):
        """Manual AP: keep base's partition entry, custom free dims."""
        return AP(base_ap.tensor, base_ap.offset + off,
                  [list(base_ap.ap[0])] + [[s, n] for s, n in dims])

    with tile.TileContext(nc) as tc:
        with ExitStack() as top:
            cpool = top.enter_context(tc.tile_pool(name="consts", bufs=1))
            kpool = top.enter_context(tc.tile_pool(name="keep", bufs=1))
            tmpp = top.enter_context(tc.tile_pool(name="tmp", bufs=1))

            x8t = [kpool.tile([P, 4 * 1536], f8, name=f"x8_{i6}")
                   for i6 in range(6)]
            nc.sync.dma_start(x8t[0][:].rearrange("p (t n) -> p t n", t=4),
                              x8_r[:, :, 0:1536])
            c_wkp = cpool.tile([P, 512], f8)
            nc.scalar.dma_start(c_wkp[:], wkp_d)
            c_bias = cpool.tile([P, 8], f32)
            nc.scalar.dma_start(c_bias[:], bias_d)
            c_dga = cpool.tile([P, 2304], f8)
            c_dgs = cpool.tile([P, 1152], f8)
            c_wvp = cpool.tile([P, 1024], f8)
            c_wq = cpool.tile([P, 1024], f8)
            c_id = cpool.tile([P, 128], bf16)
            c_sclk = cpool.tile([P, 2 * S], f32)
            c_sclv = cpool.tile([P, 4 * S], f32)
            # prewarm ACT function tables while DMAs land
            warm = cpool.tile([P, 2], f32)
            nc.scalar.activation(warm[:, 0:1], c_bias[:, 0:1], AF.Relu)
            nc.scalar.activation(warm[:, 1:2], c_bias[:, 0:1], AF.Exp)

            maps = [kpool.tile([P, MAPN], f8, name=f"map{i}") for i in range(3)]
            kbm = [kpool.tile([P, HW], bf16, name=f"kb{i}") for i in range(3)]
            rs = kpool.tile([P, 2304], bf16)
            p24 = kpool.tile([P, 6 * 576], f32)
            allp = kpool.tile([P, 6 * S], f32)
            keyn8 = kpool.tile([P, 2 * SP], f8)     # [kq, 112] padded
            valn = kpool.tile([P, 4 * S], bf16)
            vT = kpool.tile([P, 512], bf16)
            pe_all = kpool.tile([P, NCH * NCW], bf16)
            t358 = kpool.tile([P, 408], f32)

            # zero map borders: row 0, row 97, col 0, col 97
            for m in maps:
                mv = mk(m[:], MPAD, [[WP, HROWS], [1, WP]])
                nc.gpsimd.memset(mv[:, 0:1, :], 0.0)
                nc.gpsimd.memset(mv[:, 97:98, :], 0.0)
                nc.gpsimd.memset(mv[:, 1:97, 0:1], 0.0)
                nc.gpsimd.memset(mv[:, 1:97, 97:98], 0.0)
            # zero keyn pad columns (110, 111 of each kq)
            kn = keyn8[:].rearrange("p (o s) -> p o s", o=2)
            nc.gpsimd.memset(kn[:, :, S:SP], 0.0)
            nc.scalar.dma_start(c_dga[:], dga_d)
            nc.scalar.dma_start(c_dgs[:], dgs_d)
            for i6 in range(1, 6):
                nc.sync.dma_start(
                    x8t[i6][:].rearrange("p (t n) -> p t n", t=4),
                    x8_r[:, :, i6 * 1536:(i6 + 1) * 1536])
            nc.scalar.dma_start(c_wvp[:], wvp_d)
            nc.scalar.dma_start(c_wq[:], wq_d)
            nc.scalar.dma_start(c_id[:], id_d)
            nc.scalar.dma_start(c_sclk[:], sclk_d)
            nc.scalar.dma_start(c_sclv[:], sclv_d)

            def xv(lo):
                """(tile, local offset) for pixel range starting at lo."""
                t = x8t[lo // 1536]
                return t[:].rearrange("p (t n) -> p t n", t=4), lo % 1536

            wv_k = c_wkp[:].rearrange("p (kc o m) -> p kc o m", kc=2, o=2)
            wv_v = c_wvp[:].rearrange("p (kc o m) -> p kc o m", kc=2, o=2)
            wv_q = c_wq[:].rearrange("p (kc o m) -> p kc o m", kc=2, o=2)
            dgav = c_dga[:].rearrange("p (ci dx o m) -> p ci dx o m", ci=3, dx=3, o=2)
            dgsv = c_dgs[:].rearrange("p (ci dx m) -> p ci dx m", ci=3, dx=3)

            # stage-2 pools: maps [m0, m1) of p24 -> allp (4 grids each)
            def smallpools(m0, m1):
                m = m1 - m0
                allp_v = allp[:, m0 * S:m1 * S].rearrange("p (m s) -> p m s", s=S)
                p24s = p24[:, m0 * 576:m1 * 576]
                nc.vector.reduce_sum(
                    allp_v[:, :, 0:1],
                    p24s.rearrange("p (m s) -> p m s", s=576), axis=AX.X)
                tmp = tmpp.tile([P, 1152], f32, name="tmp", tag="tmp")
                nc.vector.reduce_sum(
                    tmp[:, 0:m * 72],
                    p24s.rearrange("p (mh wq ws) -> p mh wq ws", wq=3, ws=8),
                    axis=AX.X)
                nc.vector.reduce_sum(
                    allp_v[:, :, 1:10],
                    tmp[:, 0:m * 72].rearrange(
                        "p (m hq hs wq) -> p m hq wq hs", m=m, hq=3, hs=8),
                    axis=AX.X)
                tmp6 = tmpp.tile([P, 1152], f32, name="tmp6", tag="tmp")
                nc.vector.reduce_sum(
                    tmp6[:, 0:m * 144],
                    p24s.rearrange("p (mh wq ws) -> p mh wq ws", wq=6, ws=4),
                    axis=AX.X)
                nc.vector.reduce_sum(
                    allp_v[:, :, 10:46],
                    tmp6[:, 0:m * 144].rearrange(
                        "p (m hq hs wq) -> p m hq wq hs", m=m, hq=6, hs=4),
                    axis=AX.X)
                tmp8 = tmpp.tile([P, 1152], f32, name="tmp8", tag="tmp")
                nc.vector.reduce_sum(
                    tmp8[:, 0:m * 192],
                    p24s.rearrange("p (mh wq ws) -> p mh wq ws", wq=8, ws=3),
                    axis=AX.X)
                nc.vector.reduce_sum(
                    allp_v[:, :, 46:110],
                    tmp8[:, 0:m * 192].rearrange(
                        "p (m hq hs wq) -> p m hq wq hs", m=m, hq=8, hs=3),
                    axis=AX.X)

            def map_pool(mi, slot, r0, r1):
                """4x4 stage-1 pool of map mi, stage-1 rows [r0, r1)."""
                src = mk(maps[mi][:], MPAD + (4 * r0 + 1) * WP + 1,
                         [[4 * WP, r1 - r0], [4, 24], [WP, 4], [1, 4]])
                nc.vector.reduce_sum(
                    p24[:, slot * 576 + r0 * 24:slot * 576 + r1 * 24],
                    src, axis=AX.XY)

            def kb_rowsum(ci, r0, r1):
                """dw-map ci: sum 4 map-rows of each stage-1 row r0..r1 -> rs
                (bf16 all-SBUF adds run in the DVE 2x mode)."""
                n = r1 - r0
                dst = mk(rs[:], r0 * 96, [[1, n * 96]])
                a = [mk(kbm[ci][:], r0 * 384 + i * 96, [[384, n], [1, 96]])
                     for i in range(4)]
                nc.vector.tensor_tensor(dst, a[0], a[1],
                                        op=mybir.AluOpType.add)
                nc.vector.tensor_tensor(dst, dst, a[2],
                                        op=mybir.AluOpType.add)
                nc.vector.tensor_tensor(dst, dst, a[3],
                                        op=mybir.AluOpType.add)

            def kb_colred(ci, r0, r1):
                """col-reduce rs rows r0..r1 into p24 slot for dw map ci."""
                slot = (1, 4, 5)[ci]
                src = mk(rs[:], r0 * 96, [[96, r1 - r0], [4, 24], [1, 4]])
                nc.vector.reduce_sum(
                    p24[:, slot * 576 + r0 * 24:slot * 576 + r1 * 24],
                    src, axis=AX.X)

            def sp5_cols(r0, r1):
                """map slot 5: per-row column pools for grids 3/6/8."""
                sl = p24[:, 5 * 576 + r0 * 24:5 * 576 + r1 * 24]
                nc.vector.reduce_sum(
                    t358[:, r0 * 3:r1 * 3],
                    sl.rearrange("p (mh wq ws) -> p mh wq ws", wq=3, ws=8),
                    axis=AX.X)
                nc.vector.reduce_sum(
                    t358[:, 72 + r0 * 6:72 + r1 * 6],
                    sl.rearrange("p (mh wq ws) -> p mh wq ws", wq=6, ws=4),
                    axis=AX.X)
                nc.vector.reduce_sum(
                    t358[:, 216 + r0 * 8:216 + r1 * 8],
                    sl.rearrange("p (mh wq ws) -> p mh wq ws", wq=8, ws=3),
                    axis=AX.X)

            def sp5_rows_early():
                """map slot 5: row pools for hq blocks within rows 0..15/17."""
                allp_v = allp[:, 5 * S:6 * S]
                nc.vector.reduce_sum(
                    allp_v[:, 1:7],
                    t358[:, 0:48].rearrange(
                        "p (hq hs wq) -> p hq wq hs", hq=2, hs=8), axis=AX.X)
                nc.vector.reduce_sum(
                    allp_v[:, 10:34],
                    t358[:, 72:168].rearrange(
                        "p (hq hs wq) -> p hq wq hs", hq=4, hs=4), axis=AX.X)
                nc.vector.reduce_sum(
                    allp_v[:, 46:94],
                    t358[:, 216:360].rearrange(
                        "p (hq hs wq) -> p hq wq hs", hq=6, hs=3), axis=AX.X)

            def sp5_rows_late():
                """remaining hq blocks + 1x1 grid (sum of the 3x3 grid)."""
                allp_v = allp[:, 5 * S:6 * S]
                nc.vector.reduce_sum(
                    allp_v[:, 7:10],
                    t358[:, 48:72].rearrange(
                        "p (hq hs wq) -> p hq wq hs", hq=1, hs=8), axis=AX.X)
                nc.vector.reduce_sum(
                    allp_v[:, 34:46],
                    t358[:, 168:216].rearrange(
                        "p (hq hs wq) -> p hq wq hs", hq=2, hs=4), axis=AX.X)
                nc.vector.reduce_sum(
                    allp_v[:, 94:110],
                    t358[:, 360:408].rearrange(
                        "p (hq hs wq) -> p hq wq hs", hq=2, hs=3), axis=AX.X)
                nc.vector.reduce_sum(
                    allp_v[:, 0:1],
                    allp_v[:, 1:10].rearrange("p (o v) -> p o v", o=1),
                    axis=AX.X)

            def valn_mul(j):
                nc.gpsimd.tensor_mul(valn[:, j * S:(j + 1) * S],
                                     allp[:, (2 + j) * S:(3 + j) * S],
                                     c_sclv[:, j * S:(j + 1) * S])

            # ---------- primary 1x1 convs (fp8 DR), half-quad grain ----------
            # dsts: (map idx, weight view, m-slice lo, out ch tot, bias col)
            def prim_quad(mi, wv, mlo, mtot, bcol, rq, psA):
                """4 row-blocks rq*4..rq*4+3 -> one 4-bank psum + 1 ACT evac."""
                ps = psA.tile([P, 2048], f32, name="pA")
                for sub in range(4):
                    rb = rq * 4 + sub
                    xt, lo = xv(rb * 384)
                    for kc in range(2):
                        lh = wv[:, kc, :, mlo:mlo + 128] if mtot == 256 \
                            else wv[:, kc]
                        nc.tensor.matmul(
                            ps[:, sub * 512:sub * 512 + 384],
                            lh, xt[:, 2 * kc:2 * kc + 2, lo:lo + 384],
                            start=(kc == 0), stop=(kc == 1),
                            perf_mode=DR)
                dst = mk(maps[mi][:], MPAD + (rq * 16 + 1) * WP + 1,
                         [[4 * WP, 4], [WP, 4], [1, 96]])
                src = mk(ps[:], 0, [[512, 4], [96, 4], [1, 96]])
                nc.scalar.activation(dst, src, AF.Relu,
                                     bias=c_bias[:, bcol:bcol + 1])

            def dw_group(ci, g, psD):
                """2 output row-blocks of depthwise for map ci."""
                bcol = (1, 4, 5)[ci]
                m = maps[ci]
                ps = psD.tile([P, 1024], f32, name="pD", tag="pG")
                for j in range(2):
                    r0 = (g * 2 + j) * 4
                    reg = ps[:, j * 512:j * 512 + 384]
                    for t in range(6):
                        dx = t % 3
                        if t < 3:  # DR pair: dy=-1 and dy=+1
                            rhs = mk(m[:], MPAD + r0 * WP + dx,
                                     [[2 * WP, 2], [WP, 4], [1, 96]])
                            nc.tensor.matmul(reg, dgav[:, ci, dx], rhs,
                                             start=(t == 0), stop=False,
                                             perf_mode=DR)
                        else:      # single: dy=0
                            rhs = mk(m[:], MPAD + (r0 + 1) * WP + dx,
                                     [[WP, 4], [1, 96]])
                            nc.tensor.matmul(reg, dgsv[:, ci, dx], rhs,
                                             start=False, stop=(t == 5))
                bdst = mk(kbm[ci][:], g * 768, [[384, 2], [96, 4], [1, 96]])
                bsrc = mk(ps[:], 0, [[512, 2], [1, 384]])
                nc.scalar.activation(bdst, bsrc, AF.Relu,
                                     bias=c_bias[:, bcol:bcol + 1])

            def chunk_q(n, psG, qsbp):
                """Q conv for 512-px chunk n -> qsb fp8 [p, kq, 512]."""
                xt, lo = xv(n * NCW)
                qsb = qsbp.tile([P, 1024], f8, name="qsb")
                ps = psG.tile([P, 1024], f32, name="pG", tag="pG")
                for kq in range(2):
                    for kc in range(2):
                        nc.tensor.matmul(
                            ps[:, kq * 512:(kq + 1) * 512],
                            wv_q[:, kc, :, kq * 128:(kq + 1) * 128],
                            xt[:, 2 * kc:2 * kc + 2, lo:lo + NCW],
                            start=(kc == 0), stop=(kc == 1), perf_mode=DR)
                for kq in range(2):
                    nc.scalar.activation(
                        qsb[:, kq * 512:(kq + 1) * 512],
                        ps[:, kq * 512:(kq + 1) * 512], AF.Relu,
                        bias=c_bias[:, 6 + kq:7 + kq])
                return qsb

            def sim_pair(m, qs, psG):
                """simT + exp for chunks 2m, 2m+1 -> pe_all slice."""
                sps = psG.tile([P, 1024], f32, name="pG", tag="pG")
                lh = mk(keyn8[:], 0, [[SP, 2], [1, SP]])
                for h in range(2):
                    n = 2 * m + h
                    qsb = qs.pop(n)
                    rhs = mk(qsb[:], 0, [[512, 2], [1, 512]])
                    nc.tensor.matmul(sps[0:SP, h * 512:(h + 1) * 512],
                                     lh, rhs, start=True, stop=True,
                                     perf_mode=DR)
                nc.scalar.activation(pe_all[0:S, 2 * m * NCW:(2 * m + 2) * NCW],
                                     sps[0:S, :], AF.Exp)

            def vt_tp(j, psG, eng="s"):
                tp = psG.tile([P, 1024], f32, name="pG", tag="pG")
                tpv = tp[0:S, 0:64].bitcast(bf16)
                nc.tensor.transpose(tpv, valn[:, j * S:(j + 1) * S],
                                    c_id[:])
                if eng == "s":
                    nc.scalar.copy(vT[0:S, j * 128:(j + 1) * 128], tpv)
                else:
                    nc.vector.tensor_copy(vT[0:S, j * 128:(j + 1) * 128], tpv)

            nev = [0]

            def ctx_cc(m, cv, psG, cbp, eng="v"):
                """ctx matmul + evac + store for (pair m, channel block cv)."""
                T = psG.tile([P, 1024], f32, name="pG", tag="pG")
                for h in range(2):
                    nc.tensor.matmul(
                        T[:, h * 512:(h + 1) * 512],
                        vT[0:S, cv * 128:(cv + 1) * 128],
                        pe_all[0:S, (2 * m + h) * NCW:(2 * m + h + 1) * NCW],
                        start=True, stop=True)
                cb = cbp.tile([P, 1024], bf16, name="cb")
                if eng == "s":
                    nc.scalar.copy(cb[:], T[:])
                else:
                    nc.vector.tensor_copy(cb[:], T[:])
                if cv == 3:
                    dq = (nc.gpsimd, nc.scalar, nc.sync)[m % 3]
                else:
                    dq = nc.sync
                dq.dma_start(
                    ctx_r[:, cv:cv + 1, 2 * m * NCW:(2 * m + 2) * NCW],
                    cb[:].rearrange("p (c n) -> p c n", c=1))

            # ================= emission =================
            # Phase 1: k-quads interleaved with dw-k groups (fills the x-DMA
            # pipeline); DVE runs the keyn pool chain chunked alongside.
            with tc.tile_pool(name="psA", bufs=1, space="PSUM") as psA, \
                    tc.tile_pool(name="psD", bufs=2, space="PSUM") as psD:
                kq_ = lambda rq: prim_quad(0, wv_k, 0, 128, 0, rq, psA)
                dg_ = lambda g: dw_group(0, g, psD)
                kq_(0); kq_(1); dg_(0); dg_(1)
                map_pool(0, 0, 0, 6)
                kq_(2); dg_(2); dg_(3)
                kq_(3); dg_(4)
                map_pool(0, 0, 6, 12)
                dg_(5)
                kq_(4); dg_(6); dg_(7)
                map_pool(0, 0, 12, 18)
                kb_rowsum(0, 0, 8)
                kq_(5); dg_(8); dg_(9)
                map_pool(0, 0, 18, 24)
                dg_(10); dg_(11)
                kb_rowsum(0, 8, 16)
                kb_rowsum(0, 16, 24)
                kb_colred(0, 0, 24)
                smallpools(0, 2)
                nc.gpsimd.tensor_mul(kn[:, :, 0:S],
                                     allp[:, 0:2 * S].rearrange(
                                         "p (o s) -> p o s", o=2),
                                     c_sclk[:].rearrange("p (o s) -> p o s", o=2))

            # Phase 2: value primaries; first map1 pool chunks start here,
            # the rest run inside the loop's tick slots.
            with tc.tile_pool(name="psA", bufs=2, space="PSUM") as psA:
                for rq in range(6):
                    prim_quad(1, wv_v, 0, 256, 2, rq, psA)
                map_pool(1, 2, 0, 6)
                map_pool(1, 2, 6, 12)
                for rq in range(6):
                    prim_quad(2, wv_v, 128, 256, 3, rq, psA)

            # ---------- main interleave: dw-v + q/sim + ctx ----------
            with tc.tile_pool(name="psG", bufs=4, space="PSUM") as psG, \
                    tc.tile_pool(name="qsb", bufs=6) as qsbp, \
                    tc.tile_pool(name="ctxb", bufs=6) as cbp:
                NT = 24
                qs = {}
                pending = {}   # tick -> list of thunks

                def defer(t, fn):
                    pending.setdefault(t, []).append(fn)

                CV0_T, CV1_T, CV2_T = 7, 12, 18

                def side_work(t):
                    for fn in pending.pop(t, ()):
                        fn()
                    if t < 18:
                        k = t
                        qs[k] = chunk_q(k, psG, qsbp)
                        if k % 2 == 1:
                            m = k // 2
                            defer(2 * m + 3, lambda m=m: sim_pair(m, qs, psG))
                            defer(max(2 * m + 5, CV0_T),
                                  lambda m=m: ctx_cc(m, 0, psG, cbp))
                            defer(83 + m if m >= 7 else max(2 * m + 6, CV1_T + m),
                                  lambda m=m, e=("s" if m >= 7 else "v"):
                                  ctx_cc(m, 1, psG, cbp, e))
                            defer(92 + m if m >= 6 else max(2 * m + 7, CV2_T + m),
                                  lambda m=m, e=("s" if m % 2 else "v"):
                                  ctx_cc(m, 2, psG, cbp, e))

                # DVE side-chain per tick (kept small so qsb evacs stay fresh)
                dve_side = {
                    0: [lambda: map_pool(1, 2, 12, 18)],
                    1: [lambda: map_pool(1, 2, 18, 24)],
                    2: [lambda: smallpools(2, 3), lambda: valn_mul(0)],
                    3: [lambda: map_pool(2, 3, 0, 6)],
                    4: [lambda: map_pool(2, 3, 6, 12)],
                    5: [lambda: map_pool(2, 3, 12, 18),
                        lambda: kb_rowsum(1, 0, 8)],
                    6: [lambda: map_pool(2, 3, 18, 24),
                        lambda: vt_tp(0, psG)],
                    7: [lambda: smallpools(3, 4), lambda: valn_mul(1)],
                    9: [lambda: kb_rowsum(1, 8, 16)],
                    10: [lambda: kb_colred(1, 0, 16)],
                    11: [lambda: vt_tp(1, psG)],
                    12: [lambda: kb_rowsum(1, 16, 24),
                         lambda: kb_colred(1, 16, 24)],
                    13: [lambda: smallpools(4, 5), lambda: valn_mul(2)],
                    15: [lambda: vt_tp(2, psG)],
                    16: [lambda: kb_rowsum(2, 0, 8),
                         lambda: nc.sync.dma_start(pe_d[:, 0:8 * NCW],
                                                   pe_all[0:S, 0:8 * NCW])],
                    22: [lambda: kb_rowsum(2, 16, 20),
                         lambda: kb_colred(2, 16, 20),
                         lambda: sp5_cols(16, 20),
                         lambda: sp5_rows_early()],
                    20: [lambda: kb_rowsum(2, 8, 16),
                         lambda: kb_colred(2, 0, 16)],
                    21: [lambda: sp5_cols(0, 16),
                         lambda: nc.sync.dma_start(
                             pe_d[:, 8 * NCW:16 * NCW],
                             pe_all[0:S, 8 * NCW:16 * NCW])],
                }

                for t in range(NT):
                    ci = 1 if t < 12 else 2
                    dw_group(ci, t % 12, psG)
                    if t == 23:
                        kb_rowsum(2, 20, 24)
                        kb_colred(2, 20, 24)
                        sp5_cols(20, 24)
                        sp5_rows_late()
                        valn_mul(3)
                    side_work(t)
                    for fn in dve_side.get(t, ()):
                        fn()
                # flush held-back cv work as PE filler under the final
                # pool chain, then vT3 and the cv3 stream
                for t in sorted(pending):
                    for fn in pending.pop(t):
                        fn()
                vt_tp(3, psG, "v")
                for m in range(9):
                    ctx_cc(m, 3, psG, cbp, "s" if m % 2 else "v")
                nc.scalar.dma_start(pe_d[:, 16 * NCW:], pe_all[0:S, 16 * NCW:])

    nc.compile()
    return nc


def prep_host_inputs(inputs):
    """Fold BN affine into weights, quantize to fp8, build aux tensors."""
    import ml_dtypes
    F8 = ml_dtypes.float8_e4m3fn
    g = lambda a: np.ascontiguousarray(np.asarray(a, dtype=np.float32))
    wq = (g(inputs["q_g"])[:, None] * g(inputs["q_w"])[:, :, 0, 0]).T
    wkp = (g(inputs["kp_g"])[:, None] * g(inputs["kp_w"])[:, :, 0, 0]).T
    wvp = (g(inputs["vp_g"])[:, None] * g(inputs["vp_w"])[:, :, 0, 0]).T
    wkc = g(inputs["kc_g"])[:, None, None] * g(inputs["kc_w"])[:, 0]
    wvc = g(inputs["vc_g"])[:, None, None] * g(inputs["vc_w"])[:, 0]

    def pack_primary(wt):  # [512, M] -> [p, kc, o, M] flat
        m = wt.shape[1]
        return np.ascontiguousarray(
            wt.reshape(2, 2, 128, m).transpose(2, 0, 1, 3).reshape(
                128, 4 * m)).astype(F8)

    dga = np.zeros((3, 3, 128, 2, 128), np.float32)
    dgs = np.zeros((3, 3, 128, 128), np.float32)
    taps = [wkc, wvc[:128], wvc[128:]]
    for ci in range(3):
        w = taps[ci]
        for dx in range(3):
            dga[ci, dx, :, 0] = np.diag(w[:, 0, dx])
            dga[ci, dx, :, 1] = np.diag(w[:, 2, dx])
            dgs[ci, dx] = np.diag(w[:, 1, dx])

    scale110 = np.zeros(S, np.float32)
    scale110[0] = 1.0 / 9216
    scale110[1:10] = 1.0 / 1024
    scale110[10:46] = 1.0 / 256
    scale110[46:110] = 1.0 / 144
    sclk = np.broadcast_to(np.tile(scale110 / 16.0, 2), (128, 2 * S))
    sclv = np.broadcast_to(np.tile(scale110, 4), (128, 4 * S))

    bias = np.zeros((128, 8), np.float32)
    bias[:, 0] = g(inputs["kp_b"])
    bias[:, 1] = g(inputs["kc_b"])
    bias[:, 2] = g(inputs["vp_b"])[:128]
    bias[:, 3] = g(inputs["vp_b"])[128:]
    bias[:, 4] = g(inputs["vc_b"])[:128]
    bias[:, 5] = g(inputs["vc_b"])[128:]
    bias[:, 6] = g(inputs["q_b"])[:128]
    bias[:, 7] = g(inputs["q_b"])[128:]

    return {
        "wkp8": pack_primary(wkp),
        "wvp8": pack_primary(wvp),
        "wq8": pack_primary(wq),
        "dga": np.ascontiguousarray(
            dga.transpose(2, 0, 1, 3, 4).reshape(128, 2304)).astype(F8),
        "dgs": np.ascontiguousarray(
            dgs.transpose(2, 0, 1, 3).reshape(128, 1152)).astype(F8),
        "ident": np.eye(128, dtype=ml_dtypes.bfloat16),
        "sclk": np.ascontiguousarray(sclk),
        "sclv": np.ascontiguousarray(sclv),
        "bias": bias,
    }


def make_in_maps(inputs):
    import ml_dtypes
    host = prep_host_inputs(inputs)
    x = np.asarray(inputs["x"], dtype=np.float32)
    B = x.shape[0]
    in_maps = []
    for b in range(B):
        m = dict(host)
        m["x8"] = np.ascontiguousarray(x[b].reshape(512, HW)).astype(
            ml_dtypes.float8_e4m3fn)
        in_maps.append(m)
    return in_maps


_NC = None


def get_nc():
    global _NC
    if _NC is None:
        _NC = build_bass()
    return _NC


def kernel(**inputs):
    from concourse import bass_utils
    nc = get_nc()
    in_maps = make_in_maps(inputs)
    res = bass_utils.run_bass_kernel_spmd(
        nc, in_maps, core_ids=list(range(len(in_maps))), trace=False)
    x = np.asarray(inputs["x"], dtype=np.float32)
    outs = []
    for b, r in enumerate(res.results):
        ctx = r["ctx"].astype(np.float32)
        den = r["pe"].astype(np.float32).sum(axis=0, keepdims=True)
        outs.append(x[b] + (ctx / den).reshape(512, HH, HH))
    return np.stack(outs, axis=0).astype(np.float32)
